# revision 22
# baseline (speedup 1.0000x reference)
"""Binarized ResNet Bottleneck block (dense_cnn) on 8 TRN2 NeuronCores.

Math: with inference BN folded to z*s + c (s = g*rsqrt(v+eps) > 0, c = b - m*s),
binarize(htanh(bn(z))) == sign(z*s + c) == sign(z + c/s).  The whole block is:

  y1 = sign(conv1(x)  + c1/s1)                       # 1x1, 256->128
  y2 = sign(conv2(y1) + c2/s2)                       # 3x3 stride 2, pad 1
  out = sign(conv3(y2)*(s3/ss) + convsc(x) + (c3+csc)/ss)   # 1x1s + shortcut

All conv weights are binarized to +-1, so each conv is a GEMM with the pixel
dim moving on the PE.  x is pre-scaled by 1024 and split into fp16 hi+lo
(residual ~2^-22 relative; the pre-scale keeps the lo part out of the fp16
subnormal range where |x| matters).  y1/y2 are exactly +-1 in fp16.  The
shortcut weight is +-2^-10 (exact in fp16, cancels the pre-scale); conv3's
per-channel scale q = s3/ss and the bias are applied on the otherwise-idle
VectorE in f32, and each stage's binarize is one ScalarE Sign activation.

Per core (4 samples): x lives as [chan(128) x pixels(3136)] per (sample,
ktile); y1 is zero-padded to 58x58 so the 3x3 stride-2 taps become strided
access patterns.  x/out DRAM layouts are DMA-chunk-contiguous (the host
reorders) so every transfer reads/writes sequential HBM; sample 0 uses fine
chunks so the first matmul starts early.
"""

import numpy as np

import concourse.bass as bass
import concourse.tile as tile
from concourse import bacc, mybir
from concourse.bass_utils import run_bass_kernel_spmd

F16 = mybir.dt.float16
F32 = mybir.dt.float32
E4 = mybir.dt.float8e4
AF = mybir.ActivationFunctionType
ALU = mybir.AluOpType

EPS = 1e-5
NB, CIN, H, W = 32, 256, 56, 56
PLANES, OUTP = 128, 512
NCORES = 8
NPC = NB // NCORES              # samples per core
HW1 = H * W                     # 3136
WP = W + 2                      # 58 (padded row length)
HP = H + 2
HO = WO = 28
HWO = HO * WO                   # 784
PT1 = 392                       # conv1 pixel tile = 7 rows of 56
NPT1 = HW1 // PT1               # 8
PT3 = 392                       # stage2/3 pixel tile = 14 out rows of 28
SCALE = 1024.0                  # x pre-scale (power of two)

# DMA chunk plan (pixels) per sample: fine for sample 0 so conv1 starts early
CHUNKS = {0: [(i * 2 * PT1, 2 * PT1) for i in range(NPT1 // 2)]}
for _n in range(1, NPC):
    CHUNKS[_n] = [(0, 4 * PT1), (4 * PT1, 4 * PT1)]

# wts column layout (all fp16, 128 partitions = contraction dim)
_B1 = 0            # 2 ktiles x 128
_B2 = 256          # 9 taps x 128
_W3HI = 1408       # 512: hi(q * 2^g)
_W3LO = 1920       # 512: lo(q * 2^g)
_WSC = 2432        # 2 ktiles x 512: +-2^(g-10)
_WCOLS = 3456

# bias column layout ([128, 6] f32)
#   0: (c1/s1)*SCALE   1: c2/s2   2:6: cc = ((c3+csc)/ss) * 2^g
# g is a per-out-channel power-of-2 exponent keeping q*2^g in [8,16) so the
# fp16 lo part of the conv3 weights stays out of the subnormal range.


def build_bass():
    nc = bacc.Bacc("TRN2", target_bir_lowering=False, debug=False)
    nx = NPC * 2 * 128 * HW1
    xhi_d = nc.dram_tensor("xhi", [nx], F16, kind="ExternalInput")
    xlo_d = nc.dram_tensor("xlo", [nx], F16, kind="ExternalInput")
    wts_d = nc.dram_tensor("wts", [128, _WCOLS], F16, kind="ExternalInput")
    bias_d = nc.dram_tensor("bias", [128, 6], F32, kind="ExternalInput")
    out_d = nc.dram_tensor("out", [NPC * 4 * 2 * 128, PT3], E4, kind="ExternalOutput")
    warm_d = nc.dram_tensor("warm", [128, 8], F32, kind="ExternalOutput")

    with tile.TileContext(nc) as tc:
        import contextlib

        with contextlib.ExitStack() as ctx:
            const = ctx.enter_context(tc.tile_pool(name="const", bufs=1))
            xpool = ctx.enter_context(tc.tile_pool(name="x", bufs=1))
            ypool = ctx.enter_context(tc.tile_pool(name="y", bufs=1))
            opool = ctx.enter_context(tc.tile_pool(name="o", bufs=6))
            p1pool = ctx.enter_context(tc.tile_pool(name="p1", bufs=3, space="PSUM"))
            p2pool = ctx.enter_context(tc.tile_pool(name="p2", bufs=2, space="PSUM"))
            pscpool = ctx.enter_context(tc.tile_pool(name="psc", bufs=3, space="PSUM"))

            # conv1's weight blocks ship in a small separate DMA so the first
            # matmul isn't gated on the full weight transfer
            wbig = const.tile([128, _WCOLS], F16, tag="wbig")
            nc.scalar.dma_start(wbig[:, 0:256], wts_d.ap()[:, 0:256])
            nc.scalar.dma_start(wbig[:, 256:_WCOLS], wts_d.ap()[:, 256:_WCOLS])
            bias = const.tile([128, 6], F32, tag="bias")
            nc.scalar.dma_start(bias[:], bias_d.ap())

            def wslice(col):
                return wbig[:, col : col + 128]

            # PE prewarm: ~3.5us of dummy matmuls on zeros so the HAM clock
            # gate opens (1.2 -> 2.4 GHz) before the first real matmul's data
            # lands.  The escape chain (copy + tiny DMA) keeps it from DCE.
            warm = const.tile([128, 512], F16, tag="warm")
            nc.vector.memset(warm[:], 0.0)
            for r in range(8):
                pw = p1pool.tile([128, 512], F32, tag="p1", name=f"warm{r}")
                nc.tensor.matmul(
                    pw[:], warm[:, 0:128], warm[:], start=True, stop=True
                )
            wout = const.tile([128, 8], F32, tag="wout")
            nc.vector.tensor_copy(wout[:], pw[:, 0:8])
            nc.sync.dma_start(warm_d.ap(), wout[:])

            xhi = {}
            xlo = {}
            for n in range(NPC):
                for kt in range(2):
                    xhi[n, kt] = xpool.tile(
                        [128, HW1], F16, tag=f"xhi{n}{kt}", name=f"xhi{n}{kt}"
                    )
                    xlo[n, kt] = xpool.tile(
                        [128, HW1], F16, tag=f"xlo{n}{kt}", name=f"xlo{n}{kt}"
                    )

            # x DMAs: DRAM is chunk-contiguous in emission order.  Sample 0's
            # chunks issue round-robin on all three DMA-capable engines so
            # conv1's supply outruns the (HAM-cold) consumption from the start;
            # later samples stream on sync/gpsimd while scalar runs ACT.
            off = 0
            seq = 0
            for n in range(NPC):
                for p0, w in CHUNKS[n]:
                    for kt in range(2):
                        span = 128 * w
                        src_hi = xhi_d.ap()[off : off + span].rearrange(
                            "(p w) -> p w", w=w
                        )
                        src_lo = xlo_d.ap()[off : off + span].rearrange(
                            "(p w) -> p w", w=w
                        )
                        eng_hi = nc.sync
                        eng_lo = nc.gpsimd if n == 0 else nc.sync
                        eng_hi.dma_start(xhi[n, kt][:, p0 : p0 + w], src_hi)
                        eng_lo.dma_start(xlo[n, kt][:, p0 : p0 + w], src_lo)
                        off += span

            y1 = {}
            y2 = {}
            for n in range(NPC):
                y1[n] = ypool.tile([128, HP * WP], F16, tag=f"y1_{n}", name=f"y1_{n}")
                y2[n] = ypool.tile([128, HWO], F16, tag=f"y2_{n}", name=f"y2_{n}")

            def borders(n):
                v1 = y1[n][:].rearrange("p (h w) -> p h w", w=WP)
                nc.vector.memset(v1[:, 0:1, :], 0.0)
                nc.vector.memset(v1[:, HP - 1 : HP, :], 0.0)
                nc.vector.memset(v1[:, 1 : HP - 1, 0:1], 0.0)
                nc.vector.memset(v1[:, 1 : HP - 1, WP - 1 : WP], 0.0)

            def stage1(n, pts):
                v1 = y1[n][:].rearrange("p (h w) -> p h w", w=WP)
                for pt in pts:
                    p1 = p1pool.tile([128, PT1], F32, tag="p1")
                    ps = slice(pt * PT1, (pt + 1) * PT1)
                    for j, (xb, kt) in enumerate(
                        ((xhi, 0), (xhi, 1), (xlo, 0), (xlo, 1))
                    ):
                        nc.tensor.matmul(
                            p1[:],
                            wslice(_B1 + kt * 128),
                            xb[n, kt][:, ps],
                            start=(j == 0),
                            stop=(j == 3),
                        )
                    nc.scalar.activation(
                        v1[:, 7 * pt + 1 : 7 * pt + 8, 1 : 1 + W],
                        p1[:].rearrange("p (h w) -> p h w", w=W),
                        AF.Sign,
                        bias=bias[:, 0:1],
                        scale=1.0,
                    )

            def stage2(n, ht):
                v1 = y1[n][:].rearrange("p (h w) -> p h w", w=WP)
                p2 = p2pool.tile([128, PT3], F32, tag="p2", name=f"p2_{n}_{ht}")
                for tap in range(9):
                    dy, dx = divmod(tap, 3)
                    rhs = v1[:, 28 * ht + dy : 28 * ht + dy + 28 : 2, dx : dx + 56 : 2]
                    nc.tensor.matmul(
                        p2[:],
                        wslice(_B2 + tap * 128),
                        rhs,
                        start=(tap == 0),
                        stop=(tap == 8),
                    )
                nc.scalar.activation(
                    y2[n][:, ht * PT3 : (ht + 1) * PT3],
                    p2[:],
                    AF.Sign,
                    bias=bias[:, 1:2],
                    scale=1.0,
                )

            def stage3(n, ht):
                if True:
                    yslice = y2[n][:, ht * PT3 : (ht + 1) * PT3]
                    for oc in range(4):
                        psc = pscpool.tile([128, PT3], F32, tag="psc")
                        nc.tensor.matmul(
                            psc[:],
                            wslice(_W3HI + oc * 128),
                            yslice,
                            start=True,
                            stop=False,
                        )
                        nc.tensor.matmul(
                            psc[:],
                            wslice(_W3LO + oc * 128),
                            yslice,
                            start=False,
                            stop=False,
                        )
                        for kt in range(2):
                            wsc = wslice(_WSC + kt * 512 + oc * 128)
                            for j, xb in enumerate((xhi, xlo)):
                                rhs = (
                                    xb[n, kt][:]
                                    .rearrange("p (h w) -> p h w", w=W)
                                    [:, 28 * ht : 28 * ht + 28 : 2, 0:56:2]
                                )
                                nc.tensor.matmul(
                                    psc[:],
                                    wsc,
                                    rhs,
                                    start=False,
                                    stop=(kt == 1 and j == 1),
                                )
                        ot = opool.tile([128, PT3], E4, tag="ot")
                        nc.scalar.activation(
                            ot[:], psc[:], AF.Sign, bias=bias[:, 2 + oc : 3 + oc], scale=1.0
                        )
                        (nc.gpsimd if oc % 2 else nc.sync).dma_start(
                            out_d.ap()[
                                ((n * 4 + oc) * 2 + ht) * 128 : ((n * 4 + oc) * 2 + ht + 1)
                                * 128,
                                :,
                            ],
                            ot[:],
                        )

            for n in range(NPC):
                stage1(n, range(NPT1))
                borders(n)
                stage2(n, 0)
                stage2(n, 1)
                stage3(n, 0)
                stage3(n, 1)

    nc.compile()
    return nc


def _prep_inputs(x, W1, W2, W3, Wsc, g1, b1, m1, v1, g2, b2, m2, v2,
                 g3, b3, m3, v3, gs, bs, ms, vs):
    f32 = np.float32

    def sgn(w):
        return np.where(w >= 0, 1.0, -1.0).astype(f32)

    def fold(g, b, m, v):
        s = (g / np.sqrt(v + EPS)).astype(f32)
        return s, (b - m * s).astype(f32)

    s1, c1 = fold(g1, b1, m1, v1)
    s2, c2 = fold(g2, b2, m2, v2)
    s3, c3 = fold(g3, b3, m3, v3)
    ssc, csc = fold(gs, bs, ms, vs)

    wts = np.zeros((128, _WCOLS), np.float16)
    b1t = sgn(W1[:, :, 0, 0]).T                     # [256, 128]
    wts[:, _B1 : _B1 + 128] = b1t[:128]
    wts[:, _B1 + 128 : _B1 + 256] = b1t[128:]
    b2 = sgn(W2)                                    # [128, 128, 3, 3]
    for tap in range(9):
        dy, dx = divmod(tap, 3)
        wts[:, _B2 + tap * 128 : _B2 + (tap + 1) * 128] = b2[:, :, dy, dx].T
    q = (s3 / ssc).astype(f32)                      # [512]
    g = np.floor(np.log2(16.0 / q)).astype(np.int32)
    G = np.exp2(g.astype(f32)).astype(f32)          # q*G in [8, 16)
    w3 = (sgn(W3[:, :, 0, 0]).T * (q * G)[None, :]).astype(f32)   # [128, 512]
    w3hi = w3.astype(np.float16)
    w3lo = (w3 - w3hi.astype(f32)).astype(np.float16)
    wts[:, _W3HI : _W3HI + 512] = w3hi
    wts[:, _W3LO : _W3LO + 512] = w3lo
    wsc = sgn(Wsc[:, :, 0, 0]).T * (G / f32(SCALE))[None, :]  # +-2^(g-10) exact
    wts[:, _WSC : _WSC + 512] = wsc[:128].astype(np.float16)
    wts[:, _WSC + 512 : _WSC + 1024] = wsc[128:].astype(np.float16)

    bias = np.zeros((128, 6), f32)
    bias[:, 0] = (c1 / s1) * f32(SCALE)
    bias[:, 1] = c2 / s2
    bias[:, 2:6] = (((c3 + csc) / ssc) * G).reshape(4, 128).T

    xs = (x.astype(f32) * f32(SCALE)).reshape(NB, 2, 128, HW1)
    xhi = xs.astype(np.float16)
    xlo = (xs - xhi.astype(f32)).astype(np.float16)

    # chunk-contiguous per-core flat layout matching build_bass emission order
    def pack(xa):
        cores = []
        for c in range(NCORES):
            parts = []
            for n in range(NPC):
                g = xa[c * NPC + n]            # [2, 128, HW1]
                for p0, w in CHUNKS[n]:
                    for kt in range(2):
                        parts.append(g[kt, :, p0 : p0 + w].reshape(-1))
            cores.append(np.concatenate(parts))
        return cores

    return pack(xhi), pack(xlo), wts, bias


_NC_CACHE = []


def _assemble(res_results):
    outs = []
    for r in res_results:
        o = r["out"].reshape(NPC, 4, 2, 128, PT3)
        o = o.transpose(0, 1, 3, 2, 4).reshape(NPC, OUTP, HO, WO)
        outs.append(o)
    return np.concatenate(outs, axis=0).astype(np.float32)


def make_in_maps(inputs):
    xhi, xlo, wts, bias = _prep_inputs(**inputs)
    return [
        {"xhi": xhi[c], "xlo": xlo[c], "wts": wts, "bias": bias}
        for c in range(NCORES)
    ]


def kernel(**inputs):
    inputs = {k: np.asarray(v) for k, v in inputs.items()}
    in_maps = make_in_maps(inputs)
    if not _NC_CACHE:
        _NC_CACHE.append(build_bass())
    nc = _NC_CACHE[0]
    res = run_bass_kernel_spmd(nc, in_maps, core_ids=list(range(NCORES)))
    return _assemble(res.results)
